# revision 5
# baseline (speedup 1.0000x reference)
"""Based (2nd-order Taylor linear attention) Trainium2 kernel.

Problem: nn_Based_56719338111472.
  hidden [1, 512, 768] -> q,k (12 heads, f=16), v (12 heads, d=64)
  phi = 2nd-order taylor feature map (D = 1 + 16 + 256 = 273)
  causal linear attention, output projection Wo.

Key identity: phi(q)·phi(k) = 1 + (q·k)/4 + (q·k)^2/32 = 0.5 + ((q·k)+4)^2/32
so the feature map collapses to a polynomial on the plain q·k score — exact
block-causal quadratic attention with K=16 score matmuls and a Square
activation, never materializing the 273-dim features.

Sharding: head-parallel, 2 heads per core (heads 2c, 2c+1; heads >= 12 are
zero-padded virtual heads). Each core computes outT_partial = Wo_blk.T @ y_blk
(row-parallel proj_o); the host sums the per-core partials (the unshard step
for a row-parallel sharding).

Structure (vs the naive per-step version):
  - q and k projections share one matmul per e-chunk (packed lhsT, 112 rows),
    then partition-shifted evac copies split them into q_sb / k_sb.
  - den is folded into the num matmul: lhsT = [v_h | ones], so rows 0:64 of
    the accumulator are num and rows 64:128 are den (replicated); no separate
    den matmul stream.
  - inputs arrive in 5 host-prepacked [128, N] DMAs (>=960B lines); outputs
    leave in 2 DMAs with contiguous per-partition lines.
  - output projection runs in 2 column passes (cols 0:256 after block j=1,
    256:512 after j=3) so proj matmuls overlap the attention blocks.
  - element-wise work is spread: Square on ACT, mask/score-affine on Pool,
    reciprocal + divide-mul + evacs on DVE/ACT, to keep the PE stream dense
    (TRN2 PE p-state ramps 1.2 -> 2.4 GHz only under continuous execution).
"""

import math

import ml_dtypes
import numpy as np

import concourse.bass as bass
import concourse.tile as tile
from concourse import bacc, mybir
from concourse.bass import ts
from concourse.bass_utils import run_bass_kernel_spmd

# ---- problem constants (hardcoded; kernel.py must be self-contained) ----
L = 512          # sequence length
E = 768          # d_model
F = 16           # feature dim per head
HD = 64          # head dim (v)
NH = 12          # real heads
C = 128          # chunk size
NCH = L // C     # 4 chunks
ECH = E // 128   # 6 e-chunks
NCORES = 8
HPC = 2          # heads per core
NCORES_REAL = (NH + HPC - 1) // HPC

_SQ_SCALE = 1.0 / math.sqrt(32.0)
_SQ_BIAS = 4.0 / math.sqrt(32.0)

BF16 = mybir.dt.bfloat16
F32 = mybir.dt.float32


def build_kernel():
    """Build and compile the per-core Bass program (identical on all cores)."""
    nc = bacc.Bacc("TRN2", debug=False, enable_asserts=False)

    # host-prepacked DRAM blobs, all [128, N] with contiguous lines
    ht_d = nc.dram_tensor("ht", (128, ECH * L), BF16, kind="ExternalInput").ap()
    wqk_d = nc.dram_tensor("wqk", (128, ECH * 112), BF16, kind="ExternalInput").ap()
    wvm_d = nc.dram_tensor("wvm", (128, ECH * 128 + L), BF16, kind="ExternalInput").ap()
    wo_d = nc.dram_tensor("wo", (128, E), BF16, kind="ExternalInput").ap()
    outp_d = nc.dram_tensor("outp", (128, NCH, ECH, C), BF16, kind="ExternalOutput").ap()

    HB = ECH * L // 2  # half of the ht blob, 3 e-chunks

    with tile.TileContext(nc) as tc:
        with (
            tc.tile_pool(name="const", bufs=1) as const_pool,
            tc.tile_pool(name="work", bufs=1) as work,
            tc.tile_pool(name="sq_p", bufs=3) as sq_pool,
            tc.tile_pool(name="sc_p", bufs=4) as sc_pool,
            tc.tile_pool(name="ps_qk", bufs=1, space="PSUM") as ps_qk_pool,
            tc.tile_pool(name="ps_v", bufs=1, space="PSUM") as ps_v_pool,
            tc.tile_pool(name="ps_nd", bufs=1, space="PSUM") as ps_nd_pool,
            tc.tile_pool(name="ps_s", bufs=2, space="PSUM") as ps_s_pool,
            tc.tile_pool(name="ps_o", bufs=2, space="PSUM") as ps_o_pool,
        ):
            # ---- input DMAs: weights on SP queue, ht halves on ACT queue ----
            wqk_sb = const_pool.tile([128, ECH, 112], BF16, name="wqk_sb")
            nc.sync.dma_start(wqk_sb, wqk_d.rearrange("p (e c) -> p e c", e=ECH))
            ht_sb = const_pool.tile([128, ECH, L], BF16, name="ht_sb")
            ht_r = ht_d.rearrange("p (e m) -> p e m", e=ECH)
            nc.sync.dma_start(ht_sb[:, 0 : ECH // 2, :], ht_r[:, 0 : ECH // 2, :])
            wvm_sb = const_pool.tile([128, ECH * 128 + L], BF16, name="wvm_sb")
            nc.sync.dma_start(wvm_sb, wvm_d)
            wv_sb = wvm_sb[:, 0 : ECH * 128].rearrange("p (e c) -> p e c", e=ECH)
            maskx_sb = wvm_sb[:, ECH * 128 :]
            nc.sync.dma_start(
                ht_sb[:, ECH // 2 : ECH, :], ht_r[:, ECH // 2 : ECH, :]
            )
            wo_sb = const_pool.tile([128, E], BF16, name="wo_sb")
            nc.sync.dma_start(wo_sb, wo_d)

            # ---- constants; dummy act forces the LUT load to overlap DMAs ----
            sqbias_sb = const_pool.tile([128, 1], F32, name="sqbias_sb")
            nc.vector.memset(sqbias_sb, _SQ_BIAS)
            dummy_sb = const_pool.tile([1, 1], F32, name="dummy_sb")
            nc.scalar.activation(
                dummy_sb,
                sqbias_sb[0:1, :],
                mybir.ActivationFunctionType.Square,
                bias=sqbias_sb[0:1, :],
                scale=1.0,
            )

            # v_sb: per chunk j, per head h: cols 128h:128h+64 = v_h,
            # cols 128h+64:128h+128 = ones (the den rows of the lhsT)
            v_sb = work.tile([128, NCH, 2 * C], BF16, name="v_sb")
            vr = v_sb.rearrange("p j (h s) -> p j h s", h=2)
            nc.gpsimd.memset(vr[:, :, :, HD : 2 * HD], 1.0)

            # ---- merged q/k projection ----
            # lhsT cols: 0:16 q0, 32:48 q1, 64:80 k0, 96:112 k1 (32-aligned)
            ps_qk = ps_qk_pool.tile([112, L], F32, name="ps_qk")
            for e in range(ECH):
                nc.tensor.matmul(
                    ps_qk,
                    wqk_sb[:, e, :],
                    ht_sb[:, e, :],
                    start=(e == 0),
                    stop=(e == ECH - 1),
                )
            q_sb = work.tile([48, L], BF16, name="q_sb")
            nc.vector.tensor_copy(q_sb, ps_qk[0:48, :])
            k_sb = work.tile([48, L], BF16, name="k_sb")
            nc.scalar.copy(k_sb, ps_qk[64:112, :])

            # ---- v projection: v[n, d] per chunk, accumulated over e ----
            ps_v = ps_v_pool.tile([128, NCH, C], F32, name="ps_v")
            for i in range(NCH):
                for e in range(ECH):
                    nc.tensor.matmul(
                        ps_v[:, i, :],
                        ht_sb[:, e, ts(i, C)],
                        wv_sb[:, e, :],
                        start=(e == 0),
                        stop=(e == ECH - 1),
                        skip_group_check=True,
                    )
            # single strided evac into the [v | ones] layout
            nc.scalar.copy(
                vr[:, :, :, 0:HD],
                ps_v.rearrange("p j (h s) -> p j h s", h=2),
            )

            # ---- attention blocks ----
            ps_nd = [
                ps_nd_pool.tile([128, L], F32, name=f"ps_nd{h}") for h in range(HPC)
            ]

            def score_block(j):
                nj = L - C * j
                for h in range(HPC):
                    b = 32 * h
                    ps_s = ps_s_pool.tile([128, L], F32, name="ps_s", tag="blk")
                    nc.tensor.matmul(
                        ps_s[:, 0:nj],
                        k_sb[b : b + F, ts(j, C)],
                        q_sb[b : b + F, C * j : L],
                        start=True,
                        stop=True,
                    )
                    sq = sq_pool.tile([128, L], BF16, name="sq")
                    nc.scalar.activation(
                        sq[:, 0:nj],
                        ps_s[:, 0:nj],
                        mybir.ActivationFunctionType.Square,
                        bias=sqbias_sb[:, :],
                        scale=_SQ_SCALE,
                    )
                    scT = sc_pool.tile([128, L], BF16, name="scT")
                    nc.vector.scalar_tensor_tensor(
                        scT[:, 0:nj],
                        sq[:, 0:nj],
                        0.5,
                        maskx_sb[:, 0:nj],
                        op0=mybir.AluOpType.add,
                        op1=mybir.AluOpType.mult,
                    )
                    yield h, scT

            def numden_block(j, sc_tiles):
                nj = L - C * j
                for h in range(HPC):
                    nc.tensor.matmul(
                        ps_nd[h][:, C * j : L],
                        v_sb[:, j, 128 * h : 128 * h + 128],
                        sc_tiles[h][:, 0:nj],
                        start=(j == 0),
                        stop=(j == NCH - 1),
                        skip_group_check=(j != 0),
                    )

            y_sb = work.tile([128, L], BF16, name="y_sb")
            rden = [
                work.tile([128, L], F32, name=f"rden{h}") for h in range(HPC)
            ]
            o_sb = work.tile([128, NCH, ECH, C], BF16, name="o_sb")

            def out_pass(c0, c1, evac_engines):
                # columns [C*c0 : C*c1): divide + row-parallel proj + store
                sl = slice(C * c0, C * c1)
                w = C * (c1 - c0)
                for h in range(HPC):
                    nc.vector.reciprocal_approx_fast(
                        rden[h][:, sl], ps_nd[h][:, sl]
                    )
                for h in range(HPC):
                    nc.vector.tensor_tensor(
                        y_sb[64 * h : 64 * h + 64, sl],
                        ps_nd[h][0:64, sl],
                        rden[h][64:128, sl],
                        op=mybir.AluOpType.mult,
                    )
                for cc2 in range(ECH // 2):
                    ps_o = ps_o_pool.tile([128, 2, w], F32, name="ps_o", tag="o")
                    for k2 in range(2):
                        cc = 2 * cc2 + k2
                        nc.tensor.matmul(
                            ps_o[:, k2, :],
                            wo_sb[:, ts(cc, C)],
                            y_sb[:, sl],
                            start=True,
                            stop=True,
                        )
                    # evac [cc-pair, c, m'] -> o_sb[:, c0:c1, cc-pair, :]
                    eng = evac_engines[cc2]
                    copy = eng.copy if eng is nc.scalar else eng.tensor_copy
                    copy(
                        o_sb[:, c0:c1, 2 * cc2 : 2 * cc2 + 2, :].rearrange(
                            "p c k m -> p k c m"
                        ),
                        ps_o.rearrange("p k (c m) -> p k c m", c=c1 - c0),
                    )
                nc.sync.dma_start(outp_d[:, c0:c1], o_sb[:, c0:c1])

            # block pipeline: scores j ahead of numden j; output pass A
            # (cols 0:256) interleaves between numden 1 and numden 3
            sc0 = dict(score_block(0))
            sc1 = dict(score_block(1))
            numden_block(0, sc0)
            sc2 = dict(score_block(2))
            numden_block(1, sc1)
            sc3 = dict(score_block(3))
            out_pass(0, 2, [nc.scalar, nc.scalar, nc.vector])
            numden_block(2, sc2)
            numden_block(3, sc3)
            out_pass(2, 4, [nc.scalar, nc.vector, nc.scalar])

    nc.compile()
    return nc


def make_core_inputs(hidden_states, Wq, Wk, Wv, Wo):
    """Host-side marshalling: pack/cast/shard the full inputs into the
    [128, N]-line DRAM blobs the kernel expects."""
    bf16 = ml_dtypes.bfloat16
    h = np.ascontiguousarray(hidden_states[0].T).astype(np.float32)  # [768, 512]
    # ht blob [128, 6*512]: blob[p, e*512+m] = h[e*128+p, m]
    ht = np.ascontiguousarray(
        h.reshape(ECH, 128, L).transpose(1, 0, 2).reshape(128, ECH * L)
    ).astype(bf16)

    # mask extended with ones: cols 0:128 = triu (keep n <= m), 128:512 = 1
    maskx = np.ones((C, L), np.float32)
    maskx[:, 0:C] = np.triu(np.ones((C, C), np.float32))

    WqT = Wq.astype(np.float32).T  # [768, 192]
    WkT = Wk.astype(np.float32).T
    WvT = Wv.astype(np.float32).T  # [768, 768]
    in_maps = []
    for c in range(NCORES):
        wqk = np.zeros((E, 112), np.float32)  # [e, col]
        wv = np.zeros((E, 128), np.float32)
        wo = np.zeros((128, E), np.float32)
        for hh in range(HPC):
            head = HPC * c + hh
            if head >= NH:
                continue
            wqk[:, 32 * hh : 32 * hh + F] = WqT[:, F * head : F * (head + 1)]
            wqk[:, 64 + 32 * hh : 64 + 32 * hh + F] = WkT[:, F * head : F * (head + 1)]
            wv[:, 64 * hh : 64 * hh + HD] = WvT[:, HD * head : HD * (head + 1)]
            wo[64 * hh : 64 * hh + HD, :] = Wo[:, HD * head : HD * (head + 1)].T
        # [e, col] -> [128, ECH*cols] with e = ech*128 + p
        wqk_b = wqk.reshape(ECH, 128, 112).transpose(1, 0, 2).reshape(128, -1)
        wv_b = wv.reshape(ECH, 128, 128).transpose(1, 0, 2).reshape(128, -1)
        wvm = np.concatenate([wv_b, maskx], axis=1)
        in_maps.append(
            {
                "ht": ht,
                "wqk": np.ascontiguousarray(wqk_b).astype(bf16),
                "wvm": np.ascontiguousarray(wvm).astype(bf16),
                "wo": wo.astype(bf16),
            }
        )
    return in_maps


def unshard(results):
    """Sum per-core row-parallel partials and restore [1, L, E]."""
    acc = np.zeros((128, NCH, ECH, C), np.float64)
    for c in range(NCORES_REAL):
        acc += results[c]["outp"].astype(np.float64)
    # outp[p, c, cc, m'] = outT[cc*128+p, c*128+m']
    outT = acc.transpose(2, 0, 1, 3).reshape(E, L)
    return outT.T.astype(np.float32).reshape(1, L, E)


_NC_CACHE = {}


def kernel(hidden_states, Wq, Wk, Wv, Wo):
    if "nc" not in _NC_CACHE:
        _NC_CACHE["nc"] = build_kernel()
    nc = _NC_CACHE["nc"]
    in_maps = make_core_inputs(hidden_states, Wq, Wk, Wv, Wo)
    res = run_bass_kernel_spmd(nc, in_maps, core_ids=list(range(NCORES)))
    return unshard(res.results)


# revision 6
# speedup vs baseline: 1.0023x; 1.0023x over previous
"""Based (2nd-order Taylor linear attention) Trainium2 kernel.

Problem: nn_Based_56719338111472.
  hidden [1, 512, 768] -> q,k (12 heads, f=16), v (12 heads, d=64)
  phi = 2nd-order taylor feature map (D = 1 + 16 + 256 = 273)
  causal linear attention, output projection Wo.

Key identity: phi(q)·phi(k) = 1 + (q·k)/4 + (q·k)^2/32 = 0.5 + ((q·k)+4)^2/32
so the feature map collapses to a polynomial on the plain q·k score — exact
block-causal quadratic attention with K=16 score matmuls and a Square
activation, never materializing the 273-dim features.

Sharding: head-parallel, 2 heads per core (heads 2c, 2c+1; heads >= 12 are
zero-padded virtual heads). Each core computes outT_partial = Wo_blk.T @ y_blk
(row-parallel proj_o); the host sums the per-core partials (the unshard step
for a row-parallel sharding).

Structure (vs the naive per-step version):
  - q and k projections share one matmul per e-chunk (packed lhsT, 112 rows),
    then partition-shifted evac copies split them into q_sb / k_sb.
  - den is folded into the num matmul: lhsT = [v_h | ones], so rows 0:64 of
    the accumulator are num and rows 64:128 are den (replicated); no separate
    den matmul stream.
  - inputs arrive in 5 host-prepacked [128, N] DMAs (>=960B lines); outputs
    leave in 2 DMAs with contiguous per-partition lines.
  - output projection runs in 2 column passes (cols 0:256 after block j=1,
    256:512 after j=3) so proj matmuls overlap the attention blocks.
  - element-wise work is spread: Square on ACT, mask/score-affine on Pool,
    reciprocal + divide-mul + evacs on DVE/ACT, to keep the PE stream dense
    (TRN2 PE p-state ramps 1.2 -> 2.4 GHz only under continuous execution).
"""

import math

import ml_dtypes
import numpy as np

import concourse.bass as bass
import concourse.tile as tile
from concourse import bacc, mybir
from concourse.bass import ts
from concourse.bass_utils import run_bass_kernel_spmd

# ---- problem constants (hardcoded; kernel.py must be self-contained) ----
L = 512          # sequence length
E = 768          # d_model
F = 16           # feature dim per head
HD = 64          # head dim (v)
NH = 12          # real heads
C = 128          # chunk size
NCH = L // C     # 4 chunks
ECH = E // 128   # 6 e-chunks
NCORES = 8
HPC = 2          # heads per core
NCORES_REAL = (NH + HPC - 1) // HPC

_SQ_SCALE = 1.0 / math.sqrt(32.0)
_SQ_BIAS = 4.0 / math.sqrt(32.0)

BF16 = mybir.dt.bfloat16
F32 = mybir.dt.float32


def build_kernel():
    """Build and compile the per-core Bass program (identical on all cores)."""
    nc = bacc.Bacc("TRN2", debug=False, enable_asserts=False)

    # host-prepacked DRAM blobs, all [128, N] with contiguous lines
    ht_d = nc.dram_tensor("ht", (128, ECH * L), BF16, kind="ExternalInput").ap()
    wqk_d = nc.dram_tensor("wqk", (128, ECH * 112), BF16, kind="ExternalInput").ap()
    wvm_d = nc.dram_tensor("wvm", (128, ECH * 128 + L), BF16, kind="ExternalInput").ap()
    wo_d = nc.dram_tensor("wo", (128, E), BF16, kind="ExternalInput").ap()
    outp_d = nc.dram_tensor("outp", (128, NCH, ECH, C), BF16, kind="ExternalOutput").ap()

    HB = ECH * L // 2  # half of the ht blob, 3 e-chunks

    with tile.TileContext(nc) as tc:
        with (
            tc.tile_pool(name="const", bufs=1) as const_pool,
            tc.tile_pool(name="work", bufs=1) as work,
            tc.tile_pool(name="sq_p", bufs=4) as sq_pool,
            tc.tile_pool(name="sc_p", bufs=8) as sc_pool,
            tc.tile_pool(name="ps_mix", bufs=4, space="PSUM") as ps_mix_pool,
            tc.tile_pool(name="ps_vo", bufs=2, space="PSUM") as ps_vo_pool,
            tc.tile_pool(name="ps_nd", bufs=1, space="PSUM") as ps_nd_pool,
        ):
            # ---- input DMAs: wqk + ht pieces race on SP/ACT queues so the
            # q/k projection starts as early as possible ----
            wqk_sb = const_pool.tile([128, ECH, 112], BF16, name="wqk_sb")
            nc.sync.dma_start(wqk_sb, wqk_d.rearrange("p (e c) -> p e c", e=ECH))
            ht_sb = const_pool.tile([128, ECH, L], BF16, name="ht_sb")
            ht_r = ht_d.rearrange("p (e m) -> p e m", e=ECH)
            nc.scalar.dma_start(ht_sb[:, 0:2, :], ht_r[:, 0:2, :])
            nc.sync.dma_start(ht_sb[:, 2:4, :], ht_r[:, 2:4, :])
            wvm_sb = const_pool.tile([128, ECH * 128 + L], BF16, name="wvm_sb")
            nc.scalar.dma_start(wvm_sb, wvm_d)
            wv_sb = wvm_sb[:, 0 : ECH * 128].rearrange("p (e c) -> p e c", e=ECH)
            maskx_sb = wvm_sb[:, ECH * 128 :]
            nc.sync.dma_start(ht_sb[:, 4:6, :], ht_r[:, 4:6, :])
            wo_sb = const_pool.tile([128, E], BF16, name="wo_sb")
            nc.scalar.dma_start(wo_sb, wo_d)

            # ---- constants; dummy act forces the LUT load to overlap DMAs ----
            sqbias_sb = const_pool.tile([128, 1], F32, name="sqbias_sb")
            nc.vector.memset(sqbias_sb, _SQ_BIAS)
            dummy_sb = const_pool.tile([1, 1], F32, name="dummy_sb")
            nc.scalar.activation(
                dummy_sb,
                sqbias_sb[0:1, :],
                mybir.ActivationFunctionType.Square,
                bias=sqbias_sb[0:1, :],
                scale=1.0,
            )

            # v_sb: per chunk j, per head h: cols 128h:128h+64 = v_h,
            # cols 128h+64:128h+128 = ones (the den rows of the lhsT)
            v_sb = work.tile([128, NCH, 2 * C], BF16, name="v_sb")
            vr = v_sb.rearrange("p j (h s) -> p j h s", h=2)
            nc.gpsimd.memset(vr[:, :, :, HD : 2 * HD], 1.0)

            # ---- merged q/k projection ----
            # lhsT cols: 0:16 q0, 32:48 q1, 64:80 k0, 96:112 k1 (32-aligned)
            ps_qk = ps_mix_pool.tile([128, L], F32, name="blk", tag="blk")
            for e in range(ECH):
                nc.tensor.matmul(
                    ps_qk[0:112, :],
                    wqk_sb[:, e, :],
                    ht_sb[:, e, :],
                    start=(e == 0),
                    stop=(e == ECH - 1),
                )
            q_sb = work.tile([48, L], BF16, name="q_sb")
            nc.vector.tensor_copy(q_sb, ps_qk[0:48, :])
            k_sb = work.tile([48, L], BF16, name="k_sb")
            nc.scalar.copy(k_sb, ps_qk[64:112, :])

            # ---- v projection: v[n, d] per chunk, accumulated over e ----
            ps_v = ps_vo_pool.tile([128, NCH, C], F32, name="vo", tag="vo")
            for i in range(NCH):
                for e in range(ECH):
                    nc.tensor.matmul(
                        ps_v[:, i, :],
                        ht_sb[:, e, ts(i, C)],
                        wv_sb[:, e, :],
                        start=(e == 0),
                        stop=(e == ECH - 1),
                        skip_group_check=True,
                    )
            # single strided evac into the [v | ones] layout
            nc.scalar.copy(
                vr[:, :, :, 0:HD],
                ps_v.rearrange("p j (h s) -> p j h s", h=2),
            )

            # ---- attention blocks ----
            ps_nd = [
                ps_nd_pool.tile([128, L], F32, name=f"ps_nd{h}") for h in range(HPC)
            ]

            def score_block(j):
                nj = L - C * j
                for h in range(HPC):
                    b = 32 * h
                    ps_s = ps_mix_pool.tile([128, L], F32, name="blk", tag="blk")
                    nc.tensor.matmul(
                        ps_s[:, 0:nj],
                        k_sb[b : b + F, ts(j, C)],
                        q_sb[b : b + F, C * j : L],
                        start=True,
                        stop=True,
                    )
                    sq = sq_pool.tile([128, L], BF16, name="sq")
                    nc.scalar.activation(
                        sq[:, 0:nj],
                        ps_s[:, 0:nj],
                        mybir.ActivationFunctionType.Square,
                        bias=sqbias_sb[:, :],
                        scale=_SQ_SCALE,
                    )
                    scT = sc_pool.tile([128, L], BF16, name="scT")
                    nc.vector.scalar_tensor_tensor(
                        scT[:, 0:nj],
                        sq[:, 0:nj],
                        0.5,
                        maskx_sb[:, 0:nj],
                        op0=mybir.AluOpType.add,
                        op1=mybir.AluOpType.mult,
                    )
                    yield h, scT

            def numden_block(j, sc_tiles):
                nj = L - C * j
                for h in range(HPC):
                    nc.tensor.matmul(
                        ps_nd[h][:, C * j : L],
                        v_sb[:, j, 128 * h : 128 * h + 128],
                        sc_tiles[h][:, 0:nj],
                        start=(j == 0),
                        stop=(j == NCH - 1),
                        skip_group_check=(j != 0),
                    )

            y_sb = work.tile([128, L], BF16, name="y_sb")
            rden = [
                work.tile([128, L], F32, name=f"rden{h}") for h in range(HPC)
            ]
            o_sb = work.tile([128, NCH, ECH, C], BF16, name="o_sb")

            def out_pass(c0, c1, evac_engines):
                # columns [C*c0 : C*c1): divide + row-parallel proj + store
                sl = slice(C * c0, C * c1)
                w = C * (c1 - c0)
                for h in range(HPC):
                    nc.vector.reciprocal_approx_fast(
                        rden[h][:, sl], ps_nd[h][:, sl]
                    )
                for h in range(HPC):
                    nc.vector.tensor_tensor(
                        y_sb[64 * h : 64 * h + 64, sl],
                        ps_nd[h][0:64, sl],
                        rden[h][64:128, sl],
                        op=mybir.AluOpType.mult,
                    )
                for cc2 in range(ECH // 2):
                    ps_o = ps_vo_pool.tile([128, 2, w], F32, name="vo", tag="vo")
                    for k2 in range(2):
                        cc = 2 * cc2 + k2
                        nc.tensor.matmul(
                            ps_o[:, k2, :],
                            wo_sb[:, ts(cc, C)],
                            y_sb[:, sl],
                            start=True,
                            stop=True,
                        )
                    # evac [cc-pair, c, m'] -> o_sb[:, c0:c1, cc-pair, :]
                    eng = evac_engines[cc2]
                    copy = eng.copy if eng is nc.scalar else eng.tensor_copy
                    copy(
                        o_sb[:, c0:c1, 2 * cc2 : 2 * cc2 + 2, :].rearrange(
                            "p c k m -> p k c m"
                        ),
                        ps_o.rearrange("p k (c m) -> p k c m", c=c1 - c0),
                    )
                if c0 == 0:
                    nc.sync.dma_start(outp_d[:, c0:c1], o_sb[:, c0:c1])
                else:
                    nc.scalar.dma_start(outp_d[:, c0 : c0 + 1], o_sb[:, c0 : c0 + 1])
                    nc.sync.dma_start(outp_d[:, c0 + 1 : c1], o_sb[:, c0 + 1 : c1])

            # block pipeline: all 8 score streams issue up front so the PE
            # never waits on the ACT/DVE square+mask chain; output pass A
            # (cols 0:256) slots between numden 2 and numden 3
            scs = [dict(score_block(j)) for j in range(NCH)]
            numden_block(0, scs[0])
            numden_block(1, scs[1])
            numden_block(2, scs[2])
            out_pass(0, 2, [nc.scalar, nc.vector, nc.scalar])
            numden_block(3, scs[3])
            out_pass(2, 4, [nc.scalar, nc.vector, nc.scalar])

    nc.compile()
    return nc


def make_core_inputs(hidden_states, Wq, Wk, Wv, Wo):
    """Host-side marshalling: pack/cast/shard the full inputs into the
    [128, N]-line DRAM blobs the kernel expects."""
    bf16 = ml_dtypes.bfloat16
    h = np.ascontiguousarray(hidden_states[0].T).astype(np.float32)  # [768, 512]
    # ht blob [128, 6*512]: blob[p, e*512+m] = h[e*128+p, m]
    ht = np.ascontiguousarray(
        h.reshape(ECH, 128, L).transpose(1, 0, 2).reshape(128, ECH * L)
    ).astype(bf16)

    # mask extended with ones: cols 0:128 = triu (keep n <= m), 128:512 = 1
    maskx = np.ones((C, L), np.float32)
    maskx[:, 0:C] = np.triu(np.ones((C, C), np.float32))

    WqT = Wq.astype(np.float32).T  # [768, 192]
    WkT = Wk.astype(np.float32).T
    WvT = Wv.astype(np.float32).T  # [768, 768]
    in_maps = []
    for c in range(NCORES):
        wqk = np.zeros((E, 112), np.float32)  # [e, col]
        wv = np.zeros((E, 128), np.float32)
        wo = np.zeros((128, E), np.float32)
        for hh in range(HPC):
            head = HPC * c + hh
            if head >= NH:
                continue
            wqk[:, 32 * hh : 32 * hh + F] = WqT[:, F * head : F * (head + 1)]
            wqk[:, 64 + 32 * hh : 64 + 32 * hh + F] = WkT[:, F * head : F * (head + 1)]
            wv[:, 64 * hh : 64 * hh + HD] = WvT[:, HD * head : HD * (head + 1)]
            wo[64 * hh : 64 * hh + HD, :] = Wo[:, HD * head : HD * (head + 1)].T
        # [e, col] -> [128, ECH*cols] with e = ech*128 + p
        wqk_b = wqk.reshape(ECH, 128, 112).transpose(1, 0, 2).reshape(128, -1)
        wv_b = wv.reshape(ECH, 128, 128).transpose(1, 0, 2).reshape(128, -1)
        wvm = np.concatenate([wv_b, maskx], axis=1)
        in_maps.append(
            {
                "ht": ht,
                "wqk": np.ascontiguousarray(wqk_b).astype(bf16),
                "wvm": np.ascontiguousarray(wvm).astype(bf16),
                "wo": wo.astype(bf16),
            }
        )
    return in_maps


def unshard(results):
    """Sum per-core row-parallel partials and restore [1, L, E]."""
    acc = np.zeros((128, NCH, ECH, C), np.float64)
    for c in range(NCORES_REAL):
        acc += results[c]["outp"].astype(np.float64)
    # outp[p, c, cc, m'] = outT[cc*128+p, c*128+m']
    outT = acc.transpose(2, 0, 1, 3).reshape(E, L)
    return outT.T.astype(np.float32).reshape(1, L, E)


_NC_CACHE = {}


def kernel(hidden_states, Wq, Wk, Wv, Wo):
    if "nc" not in _NC_CACHE:
        _NC_CACHE["nc"] = build_kernel()
    nc = _NC_CACHE["nc"]
    in_maps = make_core_inputs(hidden_states, Wq, Wk, Wv, Wo)
    res = run_bass_kernel_spmd(nc, in_maps, core_ids=list(range(NCORES)))
    return unshard(res.results)
